# revision 28
# baseline (speedup 1.0000x reference)
"""Trainium2 Bass kernel for causal multi-head attention.

Problem: B=4, S=2048, D=1024, H=16 heads, Dh=64, fp32, causal mask.
Sharding: 8 cores = 4 batches x 2 head-groups (8 heads each). No
collectives: each core produces a partial output projection y_T
[1024, 2048] for its batch; the host sums the two head-group partials
per batch and adds the output bias.

Device-side design (per core; all matmul operands bf16 — same 1
cycle/row PE rate as fp32r but with no N>=256 restriction, half the
DMA/SBUF footprint, and 2x DVE on 16-bit ops; rel-err budget ~1e-2
vs the 2e-2 gate):
  - activations kept transposed [feature, token] for QKV:
      Q_T, K_T = W_slice^T.T @ X_T      (lhsT = W^T slices, rhs = X_T)
      V natural [token, feature]        (lhsT = X_T chunk, rhs = W_v^T)
  - scores transposed per head: S_T[k, q] = K_T_h.T @ Q_T_h (contraction
    over Dh=64 partitions); heads processed in pairs sharing one
    [128,2,512] score psum tile, one exp per pair
  - softmax: exp on ACT with the 1/sqrt(Dh) scale folded in, no
    max-subtraction (scores*scale stay within ~±4); causal handled by
    exact live-column slicing of scores/exp and a single DVE multiply
    on the 128-wide diagonal band (same f>=p triangle for every
    diagonal tile)
  - attn@V in NATURAL layout: out[q, ch] accumulated per 128-q tile as
    p2_chunk.T @ V_tile, N=65 per matmul at full bf16 rate — 128-token
    contraction AND 128 output partitions both fully used (~2.1x fewer
    PE cycles than the transposed N=512 form). V carries a 65th ones
    column so psum column 64 accumulates the softmax denominator per
    q PARTITION, making the division a cheap per-partition broadcast
    multiply on DVE (no PE broadcast matmuls needed)
  - ao transposed back for the output projection with PE
    transpose-via-identity (128 cycles per 128x128 tile)
  - output projection consumes ao_T directly; host transposes y_T back
  - cross-phase software pipelining: next q-block's QKV psum groups,
    previous pairs' transpose groups, and DEFERRED output-projection
    groups are woven between attention head pairs. Projections are
    deliberately held back to the late q-blocks, where the exp chain
    on ACT outruns the (shrunken) attention matmul stream.
"""

import numpy as np
import ml_dtypes

import concourse.tile as tile
from concourse import bacc, mybir
from concourse.bass_utils import run_bass_kernel_spmd

B = 4
S = 2048
D = 1024
H = 16
DH = 64
NCORES = 8
HPC = 8  # heads per core
C = HPC * DH  # 512 local channels per core
QB = 512  # q-block (matmul moving free dim)
NQB = S // QB  # 4
NKT = S // 128  # 16 k-tiles
SCALE = 1.0 / float(np.sqrt(DH))

F32 = mybir.dt.float32
F32R = mybir.dt.float32r
BF16 = mybir.dt.bfloat16
AF = mybir.ActivationFunctionType
ALU = mybir.AluOpType


def build_nc():
    """Build the single-core Bass program (SPMD-replicated on 8 cores)."""
    MDT = BF16

    nc = bacc.Bacc("TRN2", target_bir_lowering=False, debug=False)
    regions = []
    nc._regions = regions

    def region(name):
        regions.append((name, len(nc.inst_map)))

    xt = nc.dram_tensor("xt", [D, S], MDT, kind="ExternalInput").ap()
    wqt = nc.dram_tensor("wqt", [D, C], MDT, kind="ExternalInput").ap()
    wkt = nc.dram_tensor("wkt", [D, C], MDT, kind="ExternalInput").ap()
    wvt = nc.dram_tensor("wvt", [D, C], MDT, kind="ExternalInput").ap()
    wot = nc.dram_tensor("wot", [C, D], MDT, kind="ExternalInput").ap()
    bq_d = nc.dram_tensor("bq", [128, C // 128], F32, kind="ExternalInput").ap()
    bk_d = nc.dram_tensor("bk", [128, C // 128], F32, kind="ExternalInput").ap()
    bvb_d = nc.dram_tensor("bvb", [128, C], F32, kind="ExternalInput").ap()
    ones_d = nc.dram_tensor("ones", [128, 128], MDT, kind="ExternalInput").ap()
    # identity for the PE transposes; fp32r so the transpose psum output is
    # byte-identical to the shared [128, QB] f32 "mm" psum slots
    ident_d = nc.dram_tensor("ident", [128, 128], F32R, kind="ExternalInput").ap()
    yt = nc.dram_tensor("yt", [D, S], F32, kind="ExternalOutput").ap()

    xt_r = xt.rearrange("(mt p) s -> p mt s", p=128)

    with tile.TileContext(nc) as tc:
        with (
            tc.tile_pool(name="singles", bufs=1) as singles,
            tc.tile_pool(name="xtp", bufs=2) as xtp,
            tc.tile_pool(name="qtp", bufs=2) as qtp,
            tc.tile_pool(name="pp", bufs=4) as pp,
            tc.tile_pool(name="aonp", bufs=10) as aonp,
            tc.tile_pool(name="aotp", bufs=4) as aotp,
            tc.tile_pool(name="drp", bufs=2) as drp,
            tc.tile_pool(name="yp", bufs=4) as yp,
            tc.tile_pool(name="ps_mm", bufs=2, space="PSUM") as ps_mm,
            tc.tile_pool(name="ps_s", bufs=2, space="PSUM") as ps_s_pool,
            tc.tile_pool(name="ps_av", bufs=1, space="PSUM") as ps_av_pool,
        ):
            # ---- persistent tiles -------------------------------------
            w_q = singles.tile([128, 8, C], MDT, tag="w_q")
            w_k = singles.tile([128, 8, C], MDT, tag="w_k")
            w_v = singles.tile([128, 8, C], MDT, tag="w_v")
            w_o = singles.tile([128, 4, D], MDT, tag="w_o")
            bq_sb = singles.tile([128, C // 128], F32, tag="bq")
            bk_sb = singles.tile([128, C // 128], F32, tag="bk")
            bvb_sb = singles.tile([128, C], F32, tag="bvb")
            kt_sb = singles.tile([128, 4, S], MDT, tag="kt")
            v_sb = singles.tile([128, NKT, HPC, DH + 1], MDT, tag="v")
            ones_t = singles.tile([128, 128], MDT, tag="ones")
            ident_t = singles.tile([128, 128], F32R, tag="ident")
            masks = singles.tile([128, 2, QB], BF16, tag="masks")
            # zero operand for the per-pair psum-bank zeroing matmuls
            zer = singles.tile([128, 2, 2, DH + 1], BF16, tag="zer")

            # initial loads fan out across FOUR descriptor queues — each
            # HWDGE queue serializes at ~625ns/descriptor, so a single queue
            # would gate the prologue on descriptor processing alone
            xt_cur = xtp.tile([128, 8, QB], MDT, tag="xt")
            wq_r = wqt.rearrange("(mt p) j -> p mt j", p=128)
            wk_r = wkt.rearrange("(mt p) j -> p mt j", p=128)
            wv_r = wvt.rearrange("(mt p) j -> p mt j", p=128)
            for mt in range(8):
                nc.sync.dma_start(xt_cur[:, mt, :], xt_r[:, mt, 0:QB])
                nc.scalar.dma_start(w_q[:, mt, :], wq_r[:, mt, :])
                nc.gpsimd.dma_start(w_v[:, mt, :], wv_r[:, mt, :])
            for mt in range(8):
                q = nc.sync if mt < 4 else nc.scalar
                q.dma_start(w_k[:, mt, :], wk_r[:, mt, :])
            # small/constant inputs ride the gpsimd (SWDGE) queue
            nc.gpsimd.dma_start(bq_sb, bq_d)
            nc.gpsimd.dma_start(bk_sb, bk_d)
            nc.gpsimd.dma_start(bvb_sb, bvb_d)
            nc.gpsimd.dma_start(ones_t, ones_d)
            nc.gpsimd.dma_start(ident_t, ident_d)

            # warm-up matmuls on the small zeroes tile: its memset is the
            # very first DVE op, so these run within ~0.3us and keep the PE
            # p-state ramp warm while qkv0's matmuls are still DMA-paced
            nc.vector.memset(zer, 0.0)
            zflat0 = zer.rearrange("p a b c -> p (a b c)")
            for _ in range(6):
                ps_w = ps_mm.tile([128, QB], F32, tag="mm")
                nc.tensor.matmul(
                    ps_w[:, 0:260], zflat0[:, 0:128], zflat0, start=True, stop=True
                )
            # ones column (65th) of every per-head V block
            nc.vector.tensor_copy(
                v_sb[:, :, :, DH : DH + 1],
                ones_t.rearrange("p (a b c) -> p a b c", a=NKT, b=HPC, c=1),
            )
            # mask tile; only the [128:256] slice of row 0 is used — in
            # band-local coordinates it is the f>=p triangle that every
            # diagonal tile needs
            nc.vector.memset(masks, 1.0)
            nc.gpsimd.affine_select(
                out=masks,
                in_=masks,
                compare_op=ALU.is_ge,
                fill=0.0,
                base=-128,
                pattern=[[-256, 2], [1, QB]],
                channel_multiplier=-1,
            )
            bvb_r = bvb_sb.rearrange("p (h d) -> p h d", d=DH)

            def emit_qkv_group(qb2, xt_b, qt_b, kind, idx):
                """One psum accumulation group of the qb2 projection phase.

                kind 'q'/'k': output j-tile idx of Q_T/K_T; kind 'v': seq
                chunk idx of V. Emitted interleaved into the previous
                q-block's attention so the in-order PE stream has slack
                work during softmax dependency stalls.
                """
                qs2 = slice(qb2 * QB, (qb2 + 1) * QB)
                ps = ps_mm.tile([128, QB], F32, tag="mm")
                if kind in ("q", "k"):
                    w_sb, b_sb = (w_q, bq_sb) if kind == "q" else (w_k, bk_sb)
                    jt = idx
                    for mt in range(8):
                        nc.tensor.matmul(
                            ps,
                            w_sb[:, mt, jt * 128 : (jt + 1) * 128],
                            xt_b[:, mt, :],
                            start=(mt == 0),
                            stop=(mt == 7),
                        )
                    dst = qt_b[:, jt, :] if kind == "q" else kt_sb[:, jt, qs2]
                    nc.vector.tensor_scalar_add(dst, ps, b_sb[:, jt : jt + 1])
                else:
                    kc = idx
                    for mt in range(8):
                        nc.tensor.matmul(
                            ps,
                            xt_b[:, mt, kc * 128 : (kc + 1) * 128],
                            w_v[:, mt, :],
                            start=(mt == 0),
                            stop=(mt == 7),
                        )
                    nc.vector.tensor_tensor(
                        v_sb[:, qb2 * 4 + kc, :, 0:DH],
                        ps.rearrange("p (h d) -> p h d", d=DH),
                        bvb_r,
                        ALU.add,
                    )

            GROUPS = [("q", i) for i in range(4)] + [("k", i) for i in range(4)] + [
                ("v", i) for i in range(4)
            ]

            def make_transp_group(ao_nat_t, ao_T_t, hp):
                """Transpose one pair's natural attention output back to
                [channel, token] layout for the projection (4 128x128 PE
                transposes into one psum tile, one DVE copy out)."""

                def emit():
                    ps_tr = ps_mm.tile([128, QB], F32R, tag="mm")
                    for qt in range(4):
                        nc.tensor.transpose(
                            ps_tr[:, qt * 128 : (qt + 1) * 128],
                            ao_nat_t[:, qt, :],
                            ident_t,
                        )
                    nc.vector.tensor_copy(ao_T_t[:, hp, :], ps_tr)

                return emit

            def make_proj_group(qb, ao_T_t, et, on_act=False):
                qs2 = slice(qb * QB, (qb + 1) * QB)

                def emit():
                    ps = ps_mm.tile([128, QB], F32, tag="mm")
                    for ct in range(4):
                        nc.tensor.matmul(
                            ps,
                            w_o[:, ct, et * 128 : (et + 1) * 128],
                            ao_T_t[:, ct, :],
                            start=(ct == 0),
                            stop=(ct == 3),
                        )
                    y_t = yp.tile([128, QB], F32, tag="y")
                    if on_act:
                        nc.scalar.activation(y_t, ps, AF.Copy)
                    else:
                        nc.vector.tensor_copy(y_t, ps)
                    nc.sync.dma_start(yt[et * 128 : (et + 1) * 128, qs2], y_t)

                return emit

            # deferred-work deque: emitted between attention pairs/ktiles.
            # Transp/proj groups are deliberately drained LATE (qb2/qb3),
            # where the ACT exp chain outpaces the attention matmuls.
            fillers = []
            # how many deferred (non-qkv) fillers to pop per pair, by qb.
            # qb1/qb2 pop only the transposes they must (aonp ring deadline);
            # ALL deferred projections drain in qb3, whose exp chain on ACT
            # outruns the attention matmuls by ~20us
            POPS = {0: 0, 1: 1, 2: 1, 3: 7}

            # q-block 0: only pair 0's prerequisites up front (Q/K j-tile 0
            # and all of V); the other Q/K j-tiles are emitted at the head
            # of the pair that first needs them, so attention starts as
            # soon as the first weight tiles land
            region("qkv0")
            qt_blk = qtp.tile([128, 4, QB], MDT, tag="qt")
            for kind, idx in [("q", 0), ("k", 0), ("v", 0), ("v", 1), ("v", 2), ("v", 3)]:
                emit_qkv_group(0, xt_cur, qt_blk, kind, idx)
            qb0_rest = [("q", 1), ("k", 1), ("q", 2), ("k", 2), ("q", 3), ("k", 3)]
            xt_blk = xt_cur

            for qb in range(NQB):
                qs = slice(qb * QB, (qb + 1) * QB)
                n_kt = (qb + 1) * 4

                # stage next q-block: x prefetch + Q_T tile; its 12
                # projection groups are woven between attention pairs below
                if qb + 1 < NQB:
                    xt_next = xtp.tile([128, 8, QB], MDT, tag="xt")
                    nqs = slice((qb + 1) * QB, (qb + 2) * QB)
                    for mt in range(8):
                        nc.sync.dma_start(xt_next[:, mt, :], xt_r[:, mt, nqs])
                    qt_next = qtp.tile([128, 4, QB], MDT, tag="qt")
                    next_groups = list(GROUPS)
                else:
                    xt_next = qt_next = None
                    next_groups = []
                if qb == 0:
                    # Wo is first needed by the (deferred) proj groups —
                    # keep it behind qb1's x prefetch in the load queue
                    wo_r = wot.rearrange("(ct p) e -> p ct e", p=128)
                    for ct in range(4):
                        nc.sync.dma_start(w_o[:, ct, :], wo_r[:, ct, :])

                region(f"attn{qb}")
                ao_T_blk = aotp.tile([128, 4, QB], MDT, tag="aot")
                for hp in range(4):
                    # qb0: pair hp+1's Q/K j-tiles must be in flight before
                    # that pair's scores; emit them at this pair's head
                    if qb == 0 and hp < 3:
                        for kind, idx in qb0_rest[2 * hp : 2 * hp + 2]:
                            emit_qkv_group(0, xt_cur, qt_blk, kind, idx)
                    # per-pair filler list: 3 qkv groups of the next block
                    # (must all complete this qb) + POPS[qb] deferred
                    # transpose/projection groups, sprinkled through the
                    # kt loop so ACT always has fresh scores
                    pair_fill = []
                    for _ in range(3):
                        if next_groups:
                            kind, idx = next_groups.pop(0)
                            pair_fill.append(
                                lambda k=kind, i=idx: emit_qkv_group(
                                    qb + 1, xt_next, qt_next, k, i
                                )
                            )
                    for _ in range(POPS[qb]):
                        if fillers:
                            pair_fill.append(fillers.pop(0))

                    ps_av = ps_av_pool.tile([128, 4, 2, 128], F32, tag="av")
                    # zero the used accumulation regions of each psum bank
                    # with one matmul per bank: start=True lazily zeroes the
                    # whole 2KB zero-region, so only ONE start per bank is
                    # legal per pass; every attn@V matmul below accumulates
                    # (start=False) onto these explicit zeros. The strided
                    # out view overlaps all four (qt, hh) regions, giving
                    # the scheduler the WAW edges that order the pass.
                    zflat = zer.rearrange("p a b c -> p (a b c)")
                    for bk in range(2):
                        nc.tensor.matmul(
                            ps_av[:, 2 * bk : 2 * bk + 2, :, 0 : DH + 1],
                            zflat[:, 0:128],
                            zer,
                            start=True,
                            stop=False,
                            skip_group_check=True,
                        )
                    def emit_scores(kt):
                        """S_T[k, q] for ktile kt into a fresh psum tile."""
                        kts = slice(kt * 128, (kt + 1) * 128)
                        live0 = max(kt - qb * 4, 0) * 128
                        ps_sc = ps_s_pool.tile([128, 2, QB], F32, tag="s")
                        nc.tensor.matmul(
                            ps_sc[:, 0, live0:QB],
                            kt_sb[0:64, hp, kts],
                            qt_blk[0:64, hp, live0:QB],
                            start=True,
                            stop=True,
                        )
                        nc.tensor.matmul(
                            ps_sc[:, 1, live0:QB],
                            kt_sb[64:128, hp, kts],
                            qt_blk[64:128, hp, live0:QB],
                            start=True,
                            stop=True,
                        )
                        return ps_sc

                    # kt loop software-pipelined one stage deep: scores for
                    # kt+1 are emitted BEFORE attn@V of kt, so the PE stream
                    # always has score work in flight while ACT runs exp(kt),
                    # and the exp chain on ACT runs back-to-back
                    ps_next = emit_scores(0)
                    for kt in range(n_kt):
                        r = kt - qb * 4
                        live0 = max(r, 0) * 128
                        ps_sc = ps_next
                        if kt + 1 < n_kt:
                            ps_next = emit_scores(kt + 1)
                        p2 = pp.tile([128, 2, QB], MDT, tag="p")
                        nc.scalar.activation(
                            p2[:, :, live0:QB],
                            ps_sc[:, :, live0:QB],
                            AF.Exp,
                            scale=SCALE,
                        )
                        if r >= 0:
                            # only the 128-wide diagonal band needs masking;
                            # in band-local coords it is the same "f >= p"
                            # triangle for every r
                            band = slice(live0, live0 + 128)
                            nc.vector.tensor_tensor(
                                p2[:, :, band],
                                p2[:, :, band],
                                masks[:, 0, None, 128:256].to_broadcast(
                                    (128, 2, 128)
                                ),
                                ALU.mult,
                            )
                        if kt % 2 == 1 and pair_fill and (
                            kt != n_kt - 1 or len(pair_fill) > 1
                        ):
                            pair_fill.pop(0)()
                        # natural-layout attn@V: one matmul per live
                        # (q-tile, head): 128-token contraction, N=65
                        # (64 channels + the ones/denominator column)
                        for qt in range(max(r, 0), 4):
                            for hh in range(2):
                                nc.tensor.matmul(
                                    ps_av[:, qt, hh, 0 : DH + 1],
                                    p2[:, hh, qt * 128 : (qt + 1) * 128],
                                    v_sb[:, kt, 2 * hp + hh, :],
                                    start=False,
                                    stop=(kt == qb * 4 + qt),
                                    skip_group_check=True,
                                )

                    # softmax denominators sit in psum column 64, one per
                    # q partition: reciprocal + broadcast multiply on DVE
                    den = drp.tile([128, 4, 2, 1], F32, tag="den")
                    rec = drp.tile([128, 4, 2, 1], F32, tag="rec")
                    nc.vector.tensor_copy(den, ps_av[:, :, :, DH : DH + 1])
                    nc.vector.reciprocal(rec, den)
                    ao_nat = aonp.tile([128, 4, 128], F32R, tag="aon")
                    nc.vector.tensor_tensor(
                        ao_nat.rearrange("p a (b c) -> p a b c", b=2),
                        ps_av[:, :, :, 0:DH],
                        rec.to_broadcast((128, 4, 2, DH)),
                        ALU.mult,
                    )
                    fillers.append(make_transp_group(ao_nat, ao_T_blk, hp))

                    # remaining per-pair filler at the pair boundary
                    while pair_fill:
                        pair_fill.pop(0)()

                fillers.extend(
                    make_proj_group(qb, ao_T_blk, et, on_act=(qb == NQB - 1))
                    for et in range(8)
                )
                xt_blk = xt_next
                qt_blk = qt_next

            # drain all remaining deferred transposes/projections
            region("drain")
            while fillers:
                fillers.pop(0)()

    nc.compile()
    return nc


def make_in_maps(x, Wq_w, Wk_w, Wv_w, Wo_w, Wq_b, Wk_b, Wv_b):
    """Per-core host-side sharding + layout prep."""
    bf = ml_dtypes.bfloat16
    x = np.asarray(x, dtype=np.float32)
    ones = np.ones((128, 128), dtype=bf)
    ident = np.eye(128, dtype=np.float32)
    in_maps = []
    for c in range(NCORES):
        b, g = divmod(c, 2)
        cols = slice(g * C, (g + 1) * C)
        in_maps.append(
            {
                "xt": np.ascontiguousarray(x[b].T).astype(bf),
                "wqt": np.ascontiguousarray(np.asarray(Wq_w).T[:, cols]).astype(bf),
                "wkt": np.ascontiguousarray(np.asarray(Wk_w).T[:, cols]).astype(bf),
                "wvt": np.ascontiguousarray(np.asarray(Wv_w).T[:, cols]).astype(bf),
                "wot": np.ascontiguousarray(np.asarray(Wo_w)[:, cols].T).astype(bf),
                "bq": np.ascontiguousarray(
                    np.asarray(Wq_b, dtype=np.float32)[cols].reshape(C // 128, 128).T
                ),
                "bk": np.ascontiguousarray(
                    np.asarray(Wk_b, dtype=np.float32)[cols].reshape(C // 128, 128).T
                ),
                "bvb": np.ascontiguousarray(
                    np.tile(np.asarray(Wv_b, dtype=np.float32)[cols][None, :], (128, 1))
                ),
                "ones": ones,
                "ident": ident,
            }
        )
    return in_maps


_NC_CACHE = {}
last_results = None  # test harness reads profiling info from here


def kernel(x, mask, Wq_w, Wq_b, Wk_w, Wk_b, Wv_w, Wv_b, Wo_w, Wo_b):
    global last_results
    if "nc" not in _NC_CACHE:
        _NC_CACHE["nc"] = build_nc()
    nc = _NC_CACHE["nc"]

    in_maps = make_in_maps(x, Wq_w, Wk_w, Wv_w, Wo_w, Wq_b, Wk_b, Wv_b)
    res = run_bass_kernel_spmd(nc, in_maps, list(range(NCORES)))
    last_results = res

    bo = np.asarray(Wo_b, dtype=np.float32)
    y = np.empty((B, S, D), dtype=np.float32)
    for b in range(B):
        yt = res.results[2 * b]["yt"] + res.results[2 * b + 1]["yt"]
        y[b] = yt.T + bo[None, :]
    return y


# revision 32
# speedup vs baseline: 1.0249x; 1.0249x over previous
"""Trainium2 Bass kernel for causal multi-head attention.

Problem: B=4, S=2048, D=1024, H=16 heads, Dh=64, fp32, causal mask.
Sharding: 8 cores = 4 batches x 2 head-groups (8 heads each). No
collectives: each core produces a partial output projection y_T
[1024, 2048] for its batch; the host sums the two head-group partials
per batch and adds the output bias.

Device-side design (per core; all matmul operands bf16 — same 1
cycle/row PE rate as fp32r but with no N>=256 restriction, half the
DMA/SBUF footprint, and 2x DVE on 16-bit ops; rel-err budget ~1e-2
vs the 2e-2 gate):
  - activations kept transposed [feature, token] for QKV:
      Q_T, K_T = W_slice^T.T @ X_T      (lhsT = W^T slices, rhs = X_T)
      V natural [token, feature]        (lhsT = X_T chunk, rhs = W_v^T)
  - scores transposed per head: S_T[k, q] = K_T_h.T @ Q_T_h (contraction
    over Dh=64 partitions); heads processed in pairs sharing one
    [128,2,512] score psum tile, one exp per pair
  - softmax: exp on ACT with the 1/sqrt(Dh) scale folded in, no
    max-subtraction (scores*scale stay within ~±4); causal handled by
    exact live-column slicing of scores/exp and a single DVE multiply
    on the 128-wide diagonal band (same f>=p triangle for every
    diagonal tile)
  - attn@V in NATURAL layout: out[q, ch] accumulated per 128-q tile as
    p2_chunk.T @ V_tile, N=65 per matmul at full bf16 rate — 128-token
    contraction AND 128 output partitions both fully used (~2.1x fewer
    PE cycles than the transposed N=512 form). V carries a 65th ones
    column so psum column 64 accumulates the softmax denominator per
    q PARTITION, making the division a cheap per-partition broadcast
    multiply on DVE (no PE broadcast matmuls needed)
  - ao transposed back for the output projection with PE
    transpose-via-identity (128 cycles per 128x128 tile)
  - output projection consumes ao_T directly; host transposes y_T back
  - cross-phase software pipelining: next q-block's QKV psum groups,
    previous pairs' transpose groups, and DEFERRED output-projection
    groups are woven between attention head pairs. Projections are
    deliberately held back to the late q-blocks, where the exp chain
    on ACT outruns the (shrunken) attention matmul stream.
"""

import numpy as np
import ml_dtypes

import concourse.tile as tile
from concourse import bacc, mybir
from concourse.bass_utils import run_bass_kernel_spmd

B = 4
S = 2048
D = 1024
H = 16
DH = 64
NCORES = 8
HPC = 8  # heads per core
C = HPC * DH  # 512 local channels per core
QB = 512  # q-block (matmul moving free dim)
NQB = S // QB  # 4
NKT = S // 128  # 16 k-tiles
SCALE = 1.0 / float(np.sqrt(DH))

F32 = mybir.dt.float32
F32R = mybir.dt.float32r
BF16 = mybir.dt.bfloat16
AF = mybir.ActivationFunctionType
ALU = mybir.AluOpType


def build_nc():
    """Build the single-core Bass program (SPMD-replicated on 8 cores)."""
    MDT = BF16

    nc = bacc.Bacc("TRN2", target_bir_lowering=False, debug=False)
    regions = []
    nc._regions = regions

    def region(name):
        regions.append((name, len(nc.inst_map)))

    xt = nc.dram_tensor("xt", [D, S], MDT, kind="ExternalInput").ap()
    wqt = nc.dram_tensor("wqt", [D, C], MDT, kind="ExternalInput").ap()
    wkt = nc.dram_tensor("wkt", [D, C], MDT, kind="ExternalInput").ap()
    wvt = nc.dram_tensor("wvt", [D, C], MDT, kind="ExternalInput").ap()
    wot = nc.dram_tensor("wot", [C, D], MDT, kind="ExternalInput").ap()
    bq_d = nc.dram_tensor("bq", [128, C // 128], F32, kind="ExternalInput").ap()
    bk_d = nc.dram_tensor("bk", [128, C // 128], F32, kind="ExternalInput").ap()
    bvb_d = nc.dram_tensor("bvb", [128, C], F32, kind="ExternalInput").ap()
    ones_d = nc.dram_tensor("ones", [128, 128], MDT, kind="ExternalInput").ap()
    # identity for the PE transposes; fp32r so the transpose psum output is
    # byte-identical to the shared [128, QB] f32 "mm" psum slots
    ident_d = nc.dram_tensor("ident", [128, 128], F32R, kind="ExternalInput").ap()
    yt = nc.dram_tensor("yt", [D, S], F32, kind="ExternalOutput").ap()

    xt_r = xt.rearrange("(mt p) s -> p mt s", p=128)

    with tile.TileContext(nc) as tc:
        with (
            tc.tile_pool(name="singles", bufs=1) as singles,
            tc.tile_pool(name="xtp", bufs=2) as xtp,
            tc.tile_pool(name="qtp", bufs=2) as qtp,
            tc.tile_pool(name="pp", bufs=4) as pp,
            tc.tile_pool(name="aonp", bufs=10) as aonp,
            tc.tile_pool(name="aotp", bufs=4) as aotp,
            tc.tile_pool(name="drp", bufs=2) as drp,
            tc.tile_pool(name="yp", bufs=4) as yp,
            tc.tile_pool(name="ps_mm", bufs=2, space="PSUM") as ps_mm,
            tc.tile_pool(name="ps_s", bufs=2, space="PSUM") as ps_s_pool,
            tc.tile_pool(name="ps_av", bufs=1, space="PSUM") as ps_av_pool,
        ):
            # ---- persistent tiles -------------------------------------
            w_q = singles.tile([128, 8, C], MDT, tag="w_q")
            w_k = singles.tile([128, 8, C], MDT, tag="w_k")
            w_v = singles.tile([128, 8, C], MDT, tag="w_v")
            w_o = singles.tile([128, 4, D], MDT, tag="w_o")
            bq_sb = singles.tile([128, C // 128], F32, tag="bq")
            bk_sb = singles.tile([128, C // 128], F32, tag="bk")
            bvb_sb = singles.tile([128, C], F32, tag="bvb")
            kt_sb = singles.tile([128, 4, S], MDT, tag="kt")
            v_sb = singles.tile([128, NKT, HPC, DH + 1], MDT, tag="v")
            ones_t = singles.tile([128, 128], MDT, tag="ones")
            ident_t = singles.tile([128, 128], F32R, tag="ident")
            masks = singles.tile([128, 2, QB], BF16, tag="masks")
            # zero operand for the per-pair psum-bank zeroing matmuls
            zer = singles.tile([128, 2, 2, DH + 1], BF16, tag="zer")

            # initial loads fan out across FOUR descriptor queues — each
            # HWDGE queue serializes at ~625ns/descriptor, so a single queue
            # would gate the prologue on descriptor processing alone
            xt_cur = xtp.tile([128, 8, QB], MDT, tag="xt")
            wq_r = wqt.rearrange("(mt p) j -> p mt j", p=128)
            wk_r = wkt.rearrange("(mt p) j -> p mt j", p=128)
            wv_r = wvt.rearrange("(mt p) j -> p mt j", p=128)
            # x first, split across both HWDGE queues (each queue burns
            # ~625ns/descriptor); wq/wk follow split the same way; wv rides
            # the SWDGE queue as a third parallel channel
            for mt in range(8):
                q = nc.sync if mt % 2 == 0 else nc.scalar
                q.dma_start(xt_cur[:, mt, :], xt_r[:, mt, 0:QB])
                nc.gpsimd.dma_start(w_v[:, mt, :], wv_r[:, mt, :])
            for mt in range(8):
                q = nc.sync if mt % 2 == 0 else nc.scalar
                q.dma_start(w_q[:, mt, :], wq_r[:, mt, :])
            for mt in range(8):
                q = nc.sync if mt % 2 == 0 else nc.scalar
                q.dma_start(w_k[:, mt, :], wk_r[:, mt, :])
            # small/constant inputs ride the gpsimd (SWDGE) queue
            nc.gpsimd.dma_start(bq_sb, bq_d)
            nc.gpsimd.dma_start(bk_sb, bk_d)
            nc.gpsimd.dma_start(bvb_sb, bvb_d)
            nc.gpsimd.dma_start(ones_t, ones_d)
            nc.gpsimd.dma_start(ident_t, ident_d)

            # warm-up matmuls on the small zeroes tile: its memset is the
            # very first DVE op, so these run within ~0.3us and keep the PE
            # p-state ramp warm while qkv0's matmuls are still DMA-paced
            nc.vector.memset(zer, 0.0)
            zflat0 = zer.rearrange("p a b c -> p (a b c)")
            for _ in range(6):
                ps_w = ps_mm.tile([128, QB], F32, tag="mm")
                nc.tensor.matmul(
                    ps_w[:, 0:260], zflat0[:, 0:128], zflat0, start=True, stop=True
                )
            # ones column (65th) of every per-head V block
            nc.vector.tensor_copy(
                v_sb[:, :, :, DH : DH + 1],
                ones_t.rearrange("p (a b c) -> p a b c", a=NKT, b=HPC, c=1),
            )
            # mask tile; only the [128:256] slice of row 0 is used — in
            # band-local coordinates it is the f>=p triangle that every
            # diagonal tile needs
            nc.vector.memset(masks, 1.0)
            nc.gpsimd.affine_select(
                out=masks,
                in_=masks,
                compare_op=ALU.is_ge,
                fill=0.0,
                base=-128,
                pattern=[[-256, 2], [1, QB]],
                channel_multiplier=-1,
            )
            bvb_r = bvb_sb.rearrange("p (h d) -> p h d", d=DH)

            def emit_qkv_group(qb2, xt_b, qt_b, kind, idx):
                """One psum accumulation group of the qb2 projection phase.

                kind 'q'/'k': output j-tile idx of Q_T/K_T; kind 'v': seq
                chunk idx of V. Emitted interleaved into the previous
                q-block's attention so the in-order PE stream has slack
                work during softmax dependency stalls.
                """
                qs2 = slice(qb2 * QB, (qb2 + 1) * QB)
                ps = ps_mm.tile([128, QB], F32, tag="mm")
                if kind in ("q", "k"):
                    w_sb, b_sb = (w_q, bq_sb) if kind == "q" else (w_k, bk_sb)
                    jt = idx
                    for mt in range(8):
                        nc.tensor.matmul(
                            ps,
                            w_sb[:, mt, jt * 128 : (jt + 1) * 128],
                            xt_b[:, mt, :],
                            start=(mt == 0),
                            stop=(mt == 7),
                        )
                    dst = qt_b[:, jt, :] if kind == "q" else kt_sb[:, jt, qs2]
                    nc.vector.tensor_scalar_add(dst, ps, b_sb[:, jt : jt + 1])
                else:
                    kc = idx
                    for mt in range(8):
                        nc.tensor.matmul(
                            ps,
                            xt_b[:, mt, kc * 128 : (kc + 1) * 128],
                            w_v[:, mt, :],
                            start=(mt == 0),
                            stop=(mt == 7),
                        )
                    nc.vector.tensor_tensor(
                        v_sb[:, qb2 * 4 + kc, :, 0:DH],
                        ps.rearrange("p (h d) -> p h d", d=DH),
                        bvb_r,
                        ALU.add,
                    )

            GROUPS = [("q", i) for i in range(4)] + [("k", i) for i in range(4)] + [
                ("v", i) for i in range(4)
            ]

            def make_transp_group(ao_nat_t, ao_T_t, hp):
                """Transpose one pair's natural attention output back to
                [channel, token] layout for the projection (4 128x128 PE
                transposes into one psum tile, one DVE copy out)."""

                def emit():
                    ps_tr = ps_mm.tile([128, QB], F32R, tag="mm")
                    for qt in range(4):
                        nc.tensor.transpose(
                            ps_tr[:, qt * 128 : (qt + 1) * 128],
                            ao_nat_t[:, qt, :],
                            ident_t,
                        )
                    nc.vector.tensor_copy(ao_T_t[:, hp, :], ps_tr)

                return emit

            def make_proj_group(qb, ao_T_t, et, on_act=False):
                qs2 = slice(qb * QB, (qb + 1) * QB)

                def emit():
                    ps = ps_mm.tile([128, QB], F32, tag="mm")
                    for ct in range(4):
                        nc.tensor.matmul(
                            ps,
                            w_o[:, ct, et * 128 : (et + 1) * 128],
                            ao_T_t[:, ct, :],
                            start=(ct == 0),
                            stop=(ct == 3),
                        )
                    y_t = yp.tile([128, QB], F32, tag="y")
                    if on_act:
                        nc.scalar.activation(y_t, ps, AF.Copy)
                    else:
                        nc.vector.tensor_copy(y_t, ps)
                    nc.sync.dma_start(yt[et * 128 : (et + 1) * 128, qs2], y_t)

                return emit

            # deferred-work deque: emitted between attention pairs/ktiles.
            # Transp/proj groups are deliberately drained LATE (qb2/qb3),
            # where the ACT exp chain outpaces the attention matmuls.
            fillers = []
            # how many deferred (non-qkv) fillers to pop per pair, by qb.
            # qb1/qb2 pop only the transposes they must (aonp ring deadline);
            # ALL deferred projections drain in qb3, whose exp chain on ACT
            # outruns the attention matmuls by ~20us
            POPS = {0: 0, 1: 1, 2: 1, 3: 7}

            # q-block 0: only pair 0's prerequisites up front (Q/K j-tile 0
            # and all of V); the other Q/K j-tiles are emitted at the head
            # of the pair that first needs them, so attention starts as
            # soon as the first weight tiles land
            region("qkv0")
            qt_blk = qtp.tile([128, 4, QB], MDT, tag="qt")
            for kind, idx in [("q", 0), ("k", 0), ("v", 0), ("v", 1), ("v", 2), ("v", 3)]:
                emit_qkv_group(0, xt_cur, qt_blk, kind, idx)
            qb0_rest = [("q", 1), ("k", 1), ("q", 2), ("k", 2), ("q", 3), ("k", 3)]
            xt_blk = xt_cur

            for qb in range(NQB):
                qs = slice(qb * QB, (qb + 1) * QB)
                n_kt = (qb + 1) * 4

                # stage next q-block: x prefetch + Q_T tile; its 12
                # projection groups are woven between attention pairs below
                if qb + 1 < NQB:
                    xt_next = xtp.tile([128, 8, QB], MDT, tag="xt")
                    nqs = slice((qb + 1) * QB, (qb + 2) * QB)
                    for mt in range(8):
                        nc.sync.dma_start(xt_next[:, mt, :], xt_r[:, mt, nqs])
                    qt_next = qtp.tile([128, 4, QB], MDT, tag="qt")
                    next_groups = list(GROUPS)
                else:
                    xt_next = qt_next = None
                    next_groups = []
                if qb == 0:
                    # Wo is first needed by the (deferred) proj groups —
                    # keep it behind qb1's x prefetch in the load queue
                    wo_r = wot.rearrange("(ct p) e -> p ct e", p=128)
                    for ct in range(4):
                        nc.sync.dma_start(w_o[:, ct, :], wo_r[:, ct, :])

                region(f"attn{qb}")
                ao_T_blk = aotp.tile([128, 4, QB], MDT, tag="aot")
                for hp in range(4):
                    # qb0: pair hp+1's Q/K j-tiles must be in flight before
                    # that pair's scores; emit them at this pair's head
                    if qb == 0 and hp < 3:
                        for kind, idx in qb0_rest[2 * hp : 2 * hp + 2]:
                            emit_qkv_group(0, xt_cur, qt_blk, kind, idx)
                    # per-pair filler list: 3 qkv groups of the next block
                    # (must all complete this qb) + POPS[qb] deferred
                    # transpose/projection groups, sprinkled through the
                    # kt loop so ACT always has fresh scores
                    pair_fill = []
                    for _ in range(3):
                        if next_groups:
                            kind, idx = next_groups.pop(0)
                            pair_fill.append(
                                lambda k=kind, i=idx: emit_qkv_group(
                                    qb + 1, xt_next, qt_next, k, i
                                )
                            )
                    for _ in range(POPS[qb]):
                        if fillers:
                            pair_fill.append(fillers.pop(0))

                    ps_av = ps_av_pool.tile([128, 4, 2, 128], F32, tag="av")

                    def emit_zeroing():
                        # zero the used accumulation regions of each psum
                        # bank with one matmul per bank: start=True lazily
                        # zeroes the whole 2KB zero-region, so only ONE
                        # start per bank is legal per pass; every attn@V
                        # matmul below accumulates (start=False) onto these
                        # explicit zeros. The strided out view overlaps all
                        # four (qt, hh) regions, giving the scheduler the
                        # WAW edges that order the pass. Emitted after the
                        # first scores so its wait on the previous pair's
                        # div (DVE) hides behind the first exp latency.
                        zflat = zer.rearrange("p a b c -> p (a b c)")
                        for bk in range(2):
                            nc.tensor.matmul(
                                ps_av[:, 2 * bk : 2 * bk + 2, :, 0 : DH + 1],
                                zflat[:, 0:128],
                                zer,
                                start=True,
                                stop=False,
                                skip_group_check=True,
                            )

                    def emit_scores(kt):
                        """S_T[k, q] for ktile kt into a fresh psum tile."""
                        kts = slice(kt * 128, (kt + 1) * 128)
                        live0 = max(kt - qb * 4, 0) * 128
                        ps_sc = ps_s_pool.tile([128, 2, QB], F32, tag="s")
                        nc.tensor.matmul(
                            ps_sc[:, 0, live0:QB],
                            kt_sb[0:64, hp, kts],
                            qt_blk[0:64, hp, live0:QB],
                            start=True,
                            stop=True,
                        )
                        nc.tensor.matmul(
                            ps_sc[:, 1, live0:QB],
                            kt_sb[64:128, hp, kts],
                            qt_blk[64:128, hp, live0:QB],
                            start=True,
                            stop=True,
                        )
                        return ps_sc

                    # kt loop software-pipelined one stage deep: scores for
                    # kt+1 are emitted BEFORE attn@V of kt, so the PE stream
                    # always has score work in flight while ACT runs exp(kt),
                    # and the exp chain on ACT runs back-to-back
                    ps_next = emit_scores(0)
                    emit_zeroing()
                    for kt in range(n_kt):
                        r = kt - qb * 4
                        live0 = max(r, 0) * 128
                        ps_sc = ps_next
                        if kt + 1 < n_kt:
                            ps_next = emit_scores(kt + 1)
                        p2 = pp.tile([128, 2, QB], MDT, tag="p")
                        nc.scalar.activation(
                            p2[:, :, live0:QB],
                            ps_sc[:, :, live0:QB],
                            AF.Exp,
                            scale=SCALE,
                        )
                        if r >= 0:
                            # only the 128-wide diagonal band needs masking;
                            # in band-local coords it is the same "f >= p"
                            # triangle for every r
                            band = slice(live0, live0 + 128)
                            nc.vector.tensor_tensor(
                                p2[:, :, band],
                                p2[:, :, band],
                                masks[:, 0, None, 128:256].to_broadcast(
                                    (128, 2, 128)
                                ),
                                ALU.mult,
                            )
                        if kt % 2 == 1 and pair_fill and (
                            kt != n_kt - 1 or len(pair_fill) > 1
                        ):
                            pair_fill.pop(0)()
                        # natural-layout attn@V: one matmul per live
                        # (q-tile, head): 128-token contraction, N=65
                        # (64 channels + the ones/denominator column)
                        for qt in range(max(r, 0), 4):
                            for hh in range(2):
                                nc.tensor.matmul(
                                    ps_av[:, qt, hh, 0 : DH + 1],
                                    p2[:, hh, qt * 128 : (qt + 1) * 128],
                                    v_sb[:, kt, 2 * hp + hh, :],
                                    start=False,
                                    stop=(kt == qb * 4 + qt),
                                    skip_group_check=True,
                                )

                    # softmax denominators sit in psum column 64, one per
                    # q partition: reciprocal + broadcast multiply on DVE
                    den = drp.tile([128, 4, 2, 1], F32, tag="den")
                    rec = drp.tile([128, 4, 2, 1], F32, tag="rec")
                    nc.vector.tensor_copy(den, ps_av[:, :, :, DH : DH + 1])
                    nc.vector.reciprocal(rec, den)
                    ao_nat = aonp.tile([128, 4, 128], F32R, tag="aon")
                    nc.vector.tensor_tensor(
                        ao_nat.rearrange("p a (b c) -> p a b c", b=2),
                        ps_av[:, :, :, 0:DH],
                        rec.to_broadcast((128, 4, 2, DH)),
                        ALU.mult,
                    )
                    fillers.append(make_transp_group(ao_nat, ao_T_blk, hp))

                    # remaining per-pair filler at the pair boundary
                    while pair_fill:
                        pair_fill.pop(0)()

                # final block's projections drain after the last exp, when
                # both ACT and DVE are free: alternate the psum->sbuf copy
                # engine so the 2-buf mm ring never stalls the PE stream
                fillers.extend(
                    make_proj_group(
                        qb, ao_T_blk, et, on_act=(qb == NQB - 1 and et % 2 == 0)
                    )
                    for et in range(8)
                )
                xt_blk = xt_next
                qt_blk = qt_next

            # drain all remaining deferred transposes/projections
            region("drain")
            while fillers:
                fillers.pop(0)()

    nc.compile()
    return nc


def make_in_maps(x, Wq_w, Wk_w, Wv_w, Wo_w, Wq_b, Wk_b, Wv_b):
    """Per-core host-side sharding + layout prep."""
    bf = ml_dtypes.bfloat16
    x = np.asarray(x, dtype=np.float32)
    ones = np.ones((128, 128), dtype=bf)
    ident = np.eye(128, dtype=np.float32)
    in_maps = []
    for c in range(NCORES):
        b, g = divmod(c, 2)
        cols = slice(g * C, (g + 1) * C)
        in_maps.append(
            {
                "xt": np.ascontiguousarray(x[b].T).astype(bf),
                "wqt": np.ascontiguousarray(np.asarray(Wq_w).T[:, cols]).astype(bf),
                "wkt": np.ascontiguousarray(np.asarray(Wk_w).T[:, cols]).astype(bf),
                "wvt": np.ascontiguousarray(np.asarray(Wv_w).T[:, cols]).astype(bf),
                "wot": np.ascontiguousarray(np.asarray(Wo_w)[:, cols].T).astype(bf),
                "bq": np.ascontiguousarray(
                    np.asarray(Wq_b, dtype=np.float32)[cols].reshape(C // 128, 128).T
                ),
                "bk": np.ascontiguousarray(
                    np.asarray(Wk_b, dtype=np.float32)[cols].reshape(C // 128, 128).T
                ),
                "bvb": np.ascontiguousarray(
                    np.tile(np.asarray(Wv_b, dtype=np.float32)[cols][None, :], (128, 1))
                ),
                "ones": ones,
                "ident": ident,
            }
        )
    return in_maps


_NC_CACHE = {}
last_results = None  # test harness reads profiling info from here


def kernel(x, mask, Wq_w, Wq_b, Wk_w, Wk_b, Wv_w, Wv_b, Wo_w, Wo_b):
    global last_results
    if "nc" not in _NC_CACHE:
        _NC_CACHE["nc"] = build_nc()
    nc = _NC_CACHE["nc"]

    in_maps = make_in_maps(x, Wq_w, Wk_w, Wv_w, Wo_w, Wq_b, Wk_b, Wv_b)
    res = run_bass_kernel_spmd(nc, in_maps, list(range(NCORES)))
    last_results = res

    bo = np.asarray(Wo_b, dtype=np.float32)
    y = np.empty((B, S, D), dtype=np.float32)
    for b in range(B):
        yt = res.results[2 * b]["yt"] + res.results[2 * b + 1]["yt"]
        y[b] = yt.T + bo[None, :]
    return y
